# revision 50
# baseline (speedup 1.0000x reference)
"""Causal multi-head attention (B=4, S=2048, D=1024, H=16, hd=64) on 8 TRN2
NeuronCores.

Sharding: core c = (batch b = c//2, head-group g = c%2). Each core computes
QKV projections for its 8 heads (Megatron column-split), causal attention,
and a partial out-projection (row-split); the host sums the two head-group
partials per batch and adds the bias.

Schedule: one software-pipelined stream that keeps the PE array
continuously busy (HAM clock ramps to and stays at 2.4 GHz) and decouples
PE from the ACT engine:
  - depth-2 attention pipeline: ctx(k) issues after scores(k+2), so
    exp(k) [ACT] never gates the PE and the ACT stream runs dense
  - projection chains (V, Q/K of later head pairs, out-proj) are filler
    matmuls spread between attention matmuls (2-3 per k-tile); per-chunk
    drains enforce their deadlines
  - inputs arrive via ~20 staged DMAs of host-pre-tiled contiguous
    layouts on the sync+gpsimd queues, first-use order; masks on the
    scalar queue; exp ACT table preloaded during the DMA head
  - scalar engine runs ONLY exp; psum->sbuf copies on vector, normalize
    round trips + output on sync; hp3 processes chunks (0,1,3,2) with
    reserved out-proj fillers covering the final normalize latency
Measured: 322.9us (baseline) -> ~279us, rel err 4.0e-3 (gate 2e-2).
"""

import numpy as np
import ml_dtypes

import concourse.bass as bass
import concourse.tile as tile
from concourse import bacc, mybir
from concourse.bass_utils import run_bass_kernel_spmd

P = 128          # partitions
S = 2048         # sequence length (one batch per core)
DIN = 1024       # model dim
DG = 512         # head-group width per core (8 heads x 64)
HD = 64          # head dim
NH = 8           # heads per core
QC = 512         # q-chunk (matmul free dim)
NQC = S // QC    # 4 q-chunks
NKT = S // P     # 16 k-tiles
KDT = DIN // P   # 8 din k-tiles
NHP = 4          # head pairs per core
F32 = mybir.dt.float32
BF16 = mybir.dt.bfloat16
EXP = mybir.ActivationFunctionType.Exp

RATE_HP0 = 3     # filler matmuls per attention k-tile during head-pair 0
RATE = 2         # ... during head-pairs 1-3

_CACHE = {}


class _Chain:
    def __init__(self, name, steps):
        self.name = name
        self.steps = list(steps)
        self.i = 0

    @property
    def done(self):
        return self.i >= len(self.steps)

    def emit_next(self):
        self.steps[self.i]()
        self.i += 1


class _Filler:
    def __init__(self):
        self.order = []
        self.by_name = {}

    def add(self, chain):
        self.order.append(chain)
        self.by_name[chain.name] = chain

    def step(self, n):
        while n > 0 and self.order:
            c = self.order[0]
            if c.done:
                self.order.pop(0)
                continue
            c.emit_next()
            n -= 1

    def drain(self, name):
        c = self.by_name.get(name)
        if c is None:
            return
        while not c.done:
            c.emit_next()
        if c in self.order:
            self.order.remove(c)

    def drain_all(self):
        for c in list(self.order):
            while not c.done:
                c.emit_next()
        self.order.clear()


def _emit(tc, d):
    nc = tc.nc
    with (
        nc.allow_low_precision(reason="bf16 attention pipeline"),
        tc.tile_pool(name="persist", bufs=1) as pp,
        tc.tile_pool(name="work", bufs=4) as wp,
        tc.tile_pool(name="psc", bufs=2, space="PSUM") as psc,
        tc.tile_pool(name="ppj", bufs=2, space="PSUM") as ppj,
        tc.tile_pool(name="pcx", bufs=1, space="PSUM") as pcx,
    ):
        # ---- persistent SBUF tiles; one DMA per tile ----
        xts = [pp.tile([P, KDT, QC], BF16, tag=f"xts{s}", name=f"xts{s}") for s in range(NQC)]
        wqb = [pp.tile([P, KDT, P], BF16, tag=f"wqb{t}", name=f"wqb{t}") for t in range(NHP)]
        wkb = [pp.tile([P, KDT, P], BF16, tag=f"wkb{t}", name=f"wkb{t}") for t in range(NHP)]

        def xsl(s, j):
            return xts[s][:, j, :]
        wvt = pp.tile([P, KDT, DG], BF16, tag="wvt", name="wvt")
        wot = pp.tile([P, 4, DIN], BF16, tag="wot", name="wot")
        msk = pp.tile([P, 4, QC], BF16, tag="msk", name="msk")
        qT = [pp.tile([P, S], BF16, tag=f"qT{t}", name=f"qT{t}") for t in range(NHP)]
        kT = [pp.tile([P, S], BF16, tag=f"kT{t}", name=f"kT{t}") for t in range(NHP)]
        vv = [pp.tile([P, NH, HD + 1], BF16, tag=f"v{m}", name=f"v{m}") for m in range(NKT)]
        cx = [pp.tile([P, S], BF16, tag=f"cx{t}", name=f"cx{t}") for t in range(NHP)]

        # ---- staged input DMAs; host pre-tiles every tensor so each piece
        # is a contiguous (4-16KB/partition-run) transfer ----
        xv = d["xT"].rearrange("p (s r) -> p s r", r=KDT * QC)    # [128,4,4096]
        wqv = d["wqT"].rearrange("p (t r) -> p t r", r=KDT * P)   # [128,4,1024]
        wkv = d["wkT"].rearrange("p (t r) -> p t r", r=KDT * P)

        def xh(s, lo):
            k0, k1 = (0, KDT // 2) if lo else (KDT // 2, KDT)
            return xts[s][:, k0:k1, :]

        def xsrc(s, lo):
            r0, r1 = (0, KDT // 2 * QC) if lo else (KDT // 2 * QC, KDT * QC)
            return xv[:, s, r0:r1]

        # two streaming queues in priority order; masks alone on scalar's
        # queue (its one trigger precedes the first exp)
        nc.sync.dma_start(wqb[0][:], wqv[:, 0, :])
        nc.gpsimd.dma_start(wkb[0][:], wkv[:, 0, :])
        nc.sync.dma_start(xh(0, True), xsrc(0, True))
        nc.gpsimd.dma_start(xh(0, False), xsrc(0, False))
        nc.scalar.dma_start(msk[:], d["masks"].rearrange("p (f c) -> p f c", c=QC))
        nc.sync.dma_start(wvt[:, 0:KDT // 2, :], d["wvT"][:, 0:KDT // 2 * DG])
        nc.gpsimd.dma_start(wvt[:, KDT // 2:KDT, :], d["wvT"][:, KDT // 2 * DG:])
        nc.sync.dma_start(xh(1, True), xsrc(1, True))
        nc.gpsimd.dma_start(xh(1, False), xsrc(1, False))
        nc.sync.dma_start(xh(2, True), xsrc(2, True))
        nc.gpsimd.dma_start(xh(2, False), xsrc(2, False))
        nc.sync.dma_start(wqb[1][:], wqv[:, 1, :])
        nc.gpsimd.dma_start(wkb[1][:], wkv[:, 1, :])
        nc.sync.dma_start(xh(3, True), xsrc(3, True))
        nc.gpsimd.dma_start(xh(3, False), xsrc(3, False))
        for t in range(2, NHP):
            nc.sync.dma_start(wqb[t][:], wqv[:, t, :])
            nc.gpsimd.dma_start(wkb[t][:], wkv[:, t, :])
        nc.sync.dma_start(wot[:], d["woT"][:])

        # ones column of each v tile (sumexp lands in ctx psum row 64)
        for m in range(NKT):
            nc.vector.memset(vv[m][:, :, HD:HD + 1], 1.0)

        # preload the exp ACT table during the DMA head (the ~2.7us
        # ACT_TABLE_LOAD would otherwise land on the first chunk's
        # critical path)
        warm = wp.tile([1, 2], BF16, tag="warm", name="warm", bufs=1)
        nc.scalar.activation(warm[0:1, 0:1], vv[0][0:1, 0, HD:HD + 1], EXP, scale=0.125)

        # ---- projection chains ----
        def qk_chain(t, w, s):
            wt, dst = ((wqb, qT), (wkb, kT))[w]
            box = {}

            def mk(j):
                def go():
                    if j == 0:
                        box["ps"] = ppj.tile([P, QC], F32, tag="pj", name="ps")
                    nc.tensor.matmul(
                        box["ps"][:],
                        wt[t][:, j, :],
                        xsl(s, j),
                        start=(j == 0),
                        stop=(j == KDT - 1),
                    )
                    if j == KDT - 1:
                        # head-pair 0's copies run inside hp0 where ACT has
                        # slack and the vector FIFO is congested (see v_chain)
                        eng = nc.scalar if t == 0 else nc.vector
                        if t == 0:
                            eng.copy(dst[t][:, s * QC:(s + 1) * QC], box["ps"][:])
                        else:
                            eng.tensor_copy(dst[t][:, s * QC:(s + 1) * QC], box["ps"][:])
                return go

            return _Chain(f"qk{t}{'qk'[w]}{s}", [mk(j) for j in range(KDT)])

        def v_chain(m):
            box = {}
            sb, c0 = divmod(m, 4)

            def mk(j):
                def go():
                    if j == 0:
                        box["ps"] = ppj.tile([P, QC], F32, tag="pj", name="ps")
                    nc.tensor.matmul(
                        box["ps"][:],
                        xsl(sb, j)[:, c0 * P:(c0 + 1) * P],
                        wvt[:, j, :],
                        start=(j == 0),
                        stop=(j == KDT - 1),
                    )
                    if j == KDT - 1:
                        # scalar copy: v chains complete within head-pair 0
                        # where ACT has slack and the vector FIFO is the
                        # congested resource (delays masks/normalize)
                        nc.scalar.copy(
                            vv[m][:, :, 0:HD],
                            box["ps"][:].rearrange("p (h e) -> p h e", h=NH),
                        )
                return go

            return _Chain(f"v{m}", [mk(j) for j in range(KDT)])

        def out_chain(o, s):
            box = {}

            def mk(j):
                def go():
                    if j == 0:
                        box["ps"] = ppj.tile([P, QC], F32, tag="pj", name="ps")
                    nc.tensor.matmul(
                        box["ps"][:],
                        wot[:, j, o * P:(o + 1) * P],
                        cx[j][:, s * QC:(s + 1) * QC],
                        start=(j == 0),
                        stop=(j == 3),
                    )
                    if j == 3:
                        ob = wp.tile([P, QC], BF16, tag="ob", name="ob", bufs=4)
                        if s == 2:
                            # final chunk in hp3's (0,1,3,2) order: these
                            # copies run after the last exp, where scalar
                            # idles and vector serializes the tail
                            nc.scalar.copy(ob[:], box["ps"][:])
                        else:
                            nc.vector.tensor_copy(ob[:], box["ps"][:])
                        nc.sync.dma_start(
                            d["outT"][o * P:(o + 1) * P, s * QC:(s + 1) * QC], ob[:]
                        )
                return go

            return _Chain(f"o{o}s{s}", [mk(j) for j in range(4)])

        fill = _Filler()

        # upfront: Q/K chains for head-pair 0 chunk 0 (scores start ASAP)
        for w in range(2):
            c = qk_chain(0, w, 0)
            fill.add(c)
            fill.drain(c.name)
        # filler queue in deadline order
        for m in range(4):
            fill.add(v_chain(m))
        for s in range(1, NQC):
            for w in range(2):
                fill.add(qk_chain(0, w, s))
            for m in range(4 * s, 4 * s + 4):
                fill.add(v_chain(m))
        for t in range(1, NHP):
            for s in range(NQC):
                for w in range(2):
                    fill.add(qk_chain(t, w, s))

        # ---- attention ----
        def normalize_head(hp, s, cps):
            # rows 0:64 of cps are ctx^T, row 64 is sumexp
            cb = wp.tile([HD + 1, 2, QC], F32, tag="cb", name="cb", bufs=2)
            if (hp, s) == (NHP - 1, 2):  # final chunk: post-exp, scalar idle
                nc.scalar.copy(cb[:], cps[:])
            else:
                nc.vector.tensor_copy(cb[:], cps[:])
            # reciprocal of the [1, 1024] sumexp row with free-dim 8:
            # reshape to [128, 8] via SBUF-SBUF DMA so the DVE iterative
            # divide runs on free-dim 8 (not 1024)
            zt = wp.tile([P, 8], F32, tag="zt", name="zt", bufs=2)
            nc.gpsimd.dma_start(zt[:], cb[HD:HD + 1, :, :])
            rt = wp.tile([P, 8], F32, tag="rt", name="rt", bufs=2)
            nc.vector.reciprocal(rt[:], zt[:])
            rc = wp.tile([P, 2, QC], F32, tag="rc", name="rc", bufs=2)
            nc.gpsimd.dma_start(rc[0:1, :, :], rt[:])
            bs = wp.tile([HD, 2, QC], F32, tag="bs", name="bs", bufs=2)
            nc.gpsimd.partition_broadcast(bs[:], rc[0:1, :, :])
            return cb, bs

        def normalize_tail(hp, s, cb, bs):
            # deferred into the next chunk: keeps these vector muls from
            # blocking the next chunk's mask muls (FIFO head-of-line) and
            # gives the zt/rc/broadcast chain time to finish in background
            nc.vector.tensor_mul(
                cx[hp][0:HD, s * QC:(s + 1) * QC], cb[0:HD, 0, :], bs[:, 0, :]
            )
            cxs = wp.tile([HD, QC], BF16, tag="cxs", name="cxs", bufs=2)
            nc.vector.tensor_mul(cxs[:], cb[0:HD, 1, :], bs[:, 1, :])
            # shift partitions 0:64 -> 64:128 via SBUF DMA
            nc.gpsimd.dma_start(cx[hp][HD:P, s * QC:(s + 1) * QC], cxs[:])

        deferred = []

        def attn_chunk(hp, s, rate):
            nkt = 4 * (s + 1)  # causal: k-tiles 0..nkt-1
            cps = pcx.tile([HD + 1, 2, QC], F32, tag="cx", name="cps")
            pend = []

            def ctx_mm(k, a, s0):
                nc.tensor.matmul(
                    cps[:, 0, s0:], vv[k][:, 2 * hp, :], a[:, 0, s0:],
                    start=(k == 0), stop=(k == nkt - 1),
                )
                nc.tensor.matmul(
                    cps[:, 1, s0:], vv[k][:, 2 * hp + 1, :], a[:, 1, s0:],
                    start=(k == 0), stop=(k == nkt - 1),
                )

            for k in range(nkt):
                dd = k - 4 * s
                s0 = max(dd, 0) * P  # causal q-range restriction
                sps = psc.tile([P, 2, QC], F32, tag="sc", name="sps")
                nc.tensor.matmul(
                    sps[:, 0, s0:],
                    kT[hp][0:HD, k * P:(k + 1) * P],
                    qT[hp][0:HD, s * QC + s0:(s + 1) * QC],
                    start=True, stop=True,
                )
                nc.tensor.matmul(
                    sps[:, 1, s0:],
                    kT[hp][HD:P, k * P:(k + 1) * P],
                    qT[hp][HD:P, s * QC + s0:(s + 1) * QC],
                    start=True, stop=True,
                )
                if hp == 0:
                    fill.drain(f"v{k}")  # vv[k] needed by ctx_mm two iters on
                a = wp.tile([P, 2, QC], BF16, tag="a", name="a", bufs=12)
                nc.scalar.activation(a[:, :, s0:], sps[:, :, s0:], EXP, scale=0.125)
                if dd >= 0:
                    # only columns [s0, s0+128) straddle the diagonal
                    for h in range(2):
                        nc.vector.tensor_mul(
                            a[:, h, s0:s0 + P], a[:, h, s0:s0 + P],
                            msk[:, dd, s0:s0 + P],
                        )
                pend.append((k, a, s0))
                if len(pend) > 2:  # depth-2: ctx(k-2) after scores(k)
                    ctx_mm(*pend.pop(0))
                if k == 2 and deferred:
                    deferred.pop(0)()
                fill.step(rate)
            while pend:
                ctx_mm(*pend.pop(0))
                if pend:
                    fill.step(rate)
            cb, bs = normalize_head(hp, s, cps)
            deferred.append(
                lambda hp=hp, s=s, cb=cb, bs=bs: normalize_tail(hp, s, cb, bs)
            )

        for t in range(NHP):
            rate = RATE_HP0 if t == 0 else RATE
            # last head pair: process s=2 after s=3 so the final chunk's
            # normalize latency is covered by out-proj filler matmuls
            s_order = (0, 1, 3, 2) if t == NHP - 1 else range(NQC)
            for s in s_order:
                fill.drain(f"qk{t}q{s}")
                fill.drain(f"qk{t}k{s}")
                # reserve out(3) fillers to cover the last chunk's normalize
                attn_chunk(t, s, 1 if (t, s) == (NHP - 1, 2) else rate)
                if t == NHP - 1:
                    while deferred:  # out chains need this chunk's cx
                        deferred.pop(0)()
                    for o in range(DIN // P):
                        fill.add(out_chain(o, s))
        while deferred:
            deferred.pop(0)()
        fill.drain_all()


def _build():
    if "nc" in _CACHE:
        return _CACHE["nc"]
    nc = bacc.Bacc("TRN2", target_bir_lowering=False, debug=False, num_devices=8)
    # inputs are host-pre-tiled to [128, ...] partition-major layouts so
    # every staged DMA is a long-contiguous-run transfer
    d = {
        "xT": nc.dram_tensor("xT", [P, NQC * KDT * QC], BF16, kind="ExternalInput").ap(),
        "wqT": nc.dram_tensor("wqT", [P, NHP * KDT * P], BF16, kind="ExternalInput").ap(),
        "wkT": nc.dram_tensor("wkT", [P, NHP * KDT * P], BF16, kind="ExternalInput").ap(),
        "wvT": nc.dram_tensor("wvT", [P, KDT * DG], BF16, kind="ExternalInput").ap(),
        "woT": nc.dram_tensor("woT", [P, 4 * DIN], BF16, kind="ExternalInput").ap(),
        "masks": nc.dram_tensor("masks", [P, 4 * QC], BF16, kind="ExternalInput").ap(),
        "outT": nc.dram_tensor("outT", [DIN, S], BF16, kind="ExternalOutput").ap(),
    }
    with tile.TileContext(nc) as tc:
        _emit(tc, d)
    nc.compile()
    _CACHE["nc"] = nc
    return nc


def _masks_np():
    r = np.arange(P)[:, None]
    j = np.arange(QC)[None, :]
    return np.concatenate(
        [(j >= r + dd * P).astype(ml_dtypes.bfloat16) for dd in range(4)], axis=1
    )


def kernel(x, Wq, Wk, Wv, Wo, bo, _run_kwargs=None, _return_res=False):
    x = np.asarray(x)
    Wq, Wk, Wv, Wo, bo = (np.asarray(a) for a in (Wq, Wk, Wv, Wo, bo))
    B = x.shape[0]
    nc = _build()

    def b16(a):
        return np.ascontiguousarray(a).astype(ml_dtypes.bfloat16)

    def tile_x(xb):
        # [1024, 2048] -> (k,p,s,c) -> [p, s, k, c] -> [128, 16384]
        return np.ascontiguousarray(
            xb.T.reshape(KDT, P, NQC, QC).transpose(1, 2, 0, 3)
        ).reshape(P, -1)

    def tile_w(w):
        # [1024, 512] din-major -> [p, t, k, c] -> [128, 4096]
        return np.ascontiguousarray(
            w.reshape(KDT, P, NHP, P).transpose(1, 2, 0, 3)
        ).reshape(P, -1)

    def tile_v(w):
        # [1024, 512] -> [p, k, c] -> [128, 4096]
        return np.ascontiguousarray(w.reshape(KDT, P, DG).transpose(1, 0, 2)).reshape(P, -1)

    def tile_o(w):
        # [512, 1024] -> [p, j, c] -> [128, 4096]
        return np.ascontiguousarray(w.reshape(4, P, DIN).transpose(1, 0, 2)).reshape(P, -1)

    masks = _masks_np()
    in_maps = []
    for c in range(8):
        b, g = divmod(c, 2)
        in_maps.append({
            "xT": tile_x(b16(x[b])),
            "wqT": tile_w(b16(Wq[g * DG:(g + 1) * DG, :].T)),
            "wkT": tile_w(b16(Wk[g * DG:(g + 1) * DG, :].T)),
            "wvT": tile_v(b16(Wv[g * DG:(g + 1) * DG, :].T)),
            "woT": tile_o(b16(Wo[:, g * DG:(g + 1) * DG].T)),
            "masks": masks,
        })

    res = run_bass_kernel_spmd(nc, in_maps, list(range(8)), **(_run_kwargs or {}))
    out = np.empty((B, S, DIN), np.float32)
    for b in range(B):
        p = (res.results[2 * b]["outT"].astype(np.float32)
             + res.results[2 * b + 1]["outT"].astype(np.float32))
        out[b] = p.T + bo.astype(np.float32)
    if _return_res:
        return out, res
    return out
